# revision 31
# baseline (speedup 1.0000x reference)
"""LoRA Linear (y = x @ W^T + bias + x @ (B@A)^T) on 8 Trainium2 NeuronCores.

Strategy (column-parallel, per the out_features sharding):
  - Each core owns a 512-wide slice of out_features.
  - On device, the LoRA delta is folded into the weight once:
        W_eff^T = W_shard^T + A^T @ B_shard^T        (32 small matmuls)
    then the main GEMM runs as 64 token-tiles x 32 k-tiles of
    128x128x512 bf16 matmuls (fp32 accumulate in PSUM) with the weight
    resident in SBUF and x streamed with 8KB contiguous DMA lines.
  - All streamed operands are bf16: x/W/A/B are cast on host, y comes
    back bf16 and is cast up. This halves HBM traffic vs fp32 (x is
    replicated to all 8 cores, so it dominates) while the PE rate is
    identical to fp32r (1 row/cycle at free-size 512); accumulation
    stays fp32 so the error stays ~3e-3 relative. The PE is the hard
    floor: 128x128 MACs/cycle at ~2.37GHz effective = ~450us for the
    1.065M-row stream; fp8 modes don't raise MACs/cycle on trn2.
  - Startup is pipelined: x and bias DMAs issue on the Activation HWDGE
    queue while W streams on the SP queue (parallel DGE issue), the fold
    runs per-k-tile with its DVE adds 3-deep in PSUM, and the t=0..2
    main matmul chains trail 3 k-tiles behind the fold so the PE never
    waits on the PE->DVE->PE fold round trip. x DMAs stay 4 tiles ahead
    of use so transient HBM contention never idles (and re-throttles)
    the PE.
  - psum layout is [128 tokens, 512 out]; bias is added during PSUM
    eviction; output rows land directly in [tokens, out_shard] layout so
    the host-side gather is a plain concatenate.

Host-side work is layout only: pack x as [p, T, a, t] (so each token-tile
DMA is 128 partitions x 8KB contiguous), pre-transpose W/B slices, and
broadcast bias; then concatenate the 8 output shards.
"""

import numpy as np
from ml_dtypes import bfloat16

B_DIM, S_DIM = 4, 2048
IN_F = 4096
OUT_F = 4096
RANK = 16
N_CORES = 8
O_SHARD = OUT_F // N_CORES          # 512
TOK = B_DIM * S_DIM                 # 8192
T_TILES = TOK // 128                # 64
K_TILES = IN_F // 128               # 32
N_XBUF = 6                          # x-tile pool bufs
X_PF = 4                            # x DMA issue distance ahead of use

_CACHE = {}
LAST_RESULTS = None  # test harness introspection


def _build_nc():
    import concourse.mybir as mybir
    import concourse.tile as tile
    from concourse import bacc

    nc = bacc.Bacc("TRN2", target_bir_lowering=False)
    f32 = mybir.dt.float32
    bf16 = mybir.dt.bfloat16

    x_d = nc.dram_tensor("x_re", (128, T_TILES, K_TILES, 128), bf16,
                         kind="ExternalInput")
    w_d = nc.dram_tensor("w_re", (128, K_TILES, O_SHARD), bf16,
                         kind="ExternalInput")
    a_d = nc.dram_tensor("a_t", (RANK, IN_F), bf16, kind="ExternalInput")
    bt_d = nc.dram_tensor("b_t", (RANK, O_SHARD), bf16, kind="ExternalInput")
    bias_d = nc.dram_tensor("bias_b", (128, O_SHARD), f32,
                            kind="ExternalInput")
    y_d = nc.dram_tensor("y", (128, T_TILES // 2, 2, O_SHARD), bf16,
                         kind="ExternalOutput")

    with tile.TileContext(nc) as tc:
        with (
            tc.tile_pool(name="wpool", bufs=1) as wpool,
            tc.tile_pool(name="const", bufs=1) as const,
            tc.tile_pool(name="xpool", bufs=N_XBUF) as xpool,
            tc.tile_pool(name="opool", bufs=3) as opool,
            tc.tile_pool(name="psum", bufs=4, space="PSUM") as psum_pool,
            tc.tile_pool(name="psumF", bufs=3, space="PSUM") as psumF,
        ):
            a_sb = const.tile([RANK, IN_F], bf16)
            nc.sync.dma_start(a_sb[:], a_d[:])
            b_sb = const.tile([RANK, O_SHARD], bf16)
            nc.sync.dma_start(b_sb[:], bt_d[:])

            # Two parallel DMA issue queues: W (and y) on the SP sequencer,
            # x and bias on the Activation HWDGE. Each dma_start costs
            # ~0.6us of sequencer issue time, so splitting the streams
            # halves the serial issue latency at startup.
            N_PRE = 4
            x_tiles = [None] * T_TILES
            for t in range(N_PRE):
                x_sb = xpool.tile([128, K_TILES, 128], bf16, name=f"x{t}",
                                  tag="x")
                nc.scalar.dma_start(x_sb[:], x_d[:, t, :, :])
                x_tiles[t] = x_sb

            bias_sb = const.tile([128, O_SHARD], f32)
            nc.scalar.dma_start(bias_sb[:], bias_d[:])

            # Per-k-tile W tiles: individual tiles keep the DVE fold writes
            # and the trailing matmul reads on disjoint tiles (no aliasing
            # through a shared chunk tile's dependency tracking).
            w_sb = []
            for a in range(K_TILES):
                w_t = wpool.tile([128, O_SHARD], bf16, name=f"w{a}",
                                 tag=f"w{a}")
                nc.sync.dma_start(w_t[:], w_d[:, a, :])
                w_sb.append(w_t)

            def w_eff(a):
                return w_sb[a][:]

            # Fold the LoRA delta into W in place, with the t=0..2 main
            # matmul chains trailing LAG k-tiles behind so the PE never
            # waits on the fold's PE->DVE->PE round trip:
            #   w_eff[a] += A[:, a*128:(a+1)*128]^T @ B^T
            LAG = 4
            pt_pre = [psum_pool.tile([128, O_SHARD], f32, name=f"pt{t}",
                                     tag="pt")
                      for t in range(N_PRE)]

            def pre_mm(a):
                for t in range(N_PRE):
                    nc.tensor.matmul(
                        pt_pre[t][:],
                        x_tiles[t][:, a, :],
                        w_eff(a),
                        start=(a == 0), stop=(a == K_TILES - 1),
                    )

            for a in range(K_TILES):
                pd = psumF.tile([128, O_SHARD], f32)
                nc.tensor.matmul(
                    pd[:],
                    a_sb[:, a * 128:(a + 1) * 128],
                    b_sb[:],
                    start=True, stop=True,
                )
                nc.vector.tensor_add(w_sb[a][:], w_sb[a][:], pd[:])
                if a >= LAG:
                    pre_mm(a - LAG)
            for a in range(K_TILES - LAG, K_TILES):
                pre_mm(a)
            o2_tiles = [None] * (T_TILES // 2)

            def evict(t, pt):
                P = t // 2
                if o2_tiles[P] is None:
                    o2_tiles[P] = opool.tile([128, 2, O_SHARD], bf16,
                                             name="o2", tag="o")
                nc.vector.tensor_add(o2_tiles[P][:, t % 2, :], pt[:],
                                     bias_sb[:])
                if t % 2 == 1:
                    nc.sync.dma_start(y_d[:, P, :, :], o2_tiles[P][:])

            for t in range(N_PRE):
                evict(t, pt_pre[t])

            # Main GEMM: psum[128t, 512o] = sum_a x_tile_a^T @ w_eff_a
            # x DMAs issue X_PF tiles ahead of use so a transient HBM
            # hiccup never idles the PE long enough to re-throttle HAM.
            for t in range(N_PRE, T_TILES):
                for tp in range(t, min(t + X_PF, T_TILES)):
                    if x_tiles[tp] is None:
                        x_sb = xpool.tile([128, K_TILES, 128], bf16,
                                          name="x", tag="x")
                        nc.scalar.dma_start(x_sb[:], x_d[:, tp, :, :])
                        x_tiles[tp] = x_sb
                pt = psum_pool.tile([128, O_SHARD], f32, name="pt",
                                    tag="pt")
                for a in range(K_TILES):
                    nc.tensor.matmul(
                        pt[:],
                        x_tiles[t][:, a, :],
                        w_eff(a),
                        start=(a == 0), stop=(a == K_TILES - 1),
                    )
                evict(t, pt)

    nc.compile()
    return nc


def _pack_x(x):
    x2 = np.asarray(x, dtype=np.float32).reshape(TOK, IN_F).astype(bfloat16)
    # x_re[p, T, a, t] = x2[T*128 + t, a*128 + p]
    xr = x2.reshape(T_TILES, 128, K_TILES, 128)      # (T, t, a, p)
    return np.ascontiguousarray(xr.transpose(3, 0, 2, 1))


def kernel(x, weight, A, B, bias):
    global LAST_RESULTS
    from concourse.bass_utils import run_bass_kernel_spmd

    if "nc" not in _CACHE:
        _CACHE["nc"] = _build_nc()
    nc = _CACHE["nc"]

    weight = np.asarray(weight, dtype=np.float32)
    A = np.asarray(A, dtype=np.float32)
    B = np.asarray(B, dtype=np.float32)
    bias = np.asarray(bias, dtype=np.float32)

    x_re = _pack_x(x)
    a_t = np.ascontiguousarray(A.astype(bfloat16))

    in_maps = []
    for c in range(N_CORES):
        sl = slice(c * O_SHARD, (c + 1) * O_SHARD)
        w_s = weight[sl].astype(bfloat16)             # (512, 4096)
        # w_re[p, a, o] = w_s[o, a*128 + p]
        w_re = np.ascontiguousarray(
            w_s.T.reshape(K_TILES, 128, O_SHARD).transpose(1, 0, 2))
        b_t = np.ascontiguousarray(B[sl].T.astype(bfloat16))   # (16, 512)
        bias_b = np.ascontiguousarray(
            np.broadcast_to(bias[sl], (128, O_SHARD)))
        in_maps.append({
            "x_re": x_re,
            "w_re": w_re,
            "a_t": a_t,
            "b_t": b_t,
            "bias_b": bias_b,
        })

    res = run_bass_kernel_spmd(nc, in_maps, core_ids=list(range(N_CORES)))
    LAST_RESULTS = res

    y = np.concatenate(
        [res.results[c]["y"].transpose(1, 2, 0, 3).reshape(TOK, O_SHARD)
         .astype(np.float32)
         for c in range(N_CORES)],
        axis=1)
    return y.reshape(B_DIM, S_DIM, OUT_F)


# revision 33
# speedup vs baseline: 1.0097x; 1.0097x over previous
"""LoRA Linear (y = x @ W^T + bias + x @ (B@A)^T) on 8 Trainium2 NeuronCores.

Strategy (column-parallel, per the out_features sharding):
  - Each core owns a 512-wide slice of out_features.
  - On device, the LoRA delta is folded into the weight once:
        W_eff^T = W_shard^T + A^T @ B_shard^T        (32 small matmuls)
    then the main GEMM runs as 64 token-tiles x 32 k-tiles of
    128x128x512 bf16 matmuls (fp32 accumulate in PSUM) with the weight
    resident in SBUF and x streamed with 8KB contiguous DMA lines.
  - All streamed operands are bf16: x/W/A/B are cast on host, y comes
    back bf16 and is cast up. This halves HBM traffic vs fp32 (x is
    replicated to all 8 cores, so it dominates) while the PE rate is
    identical to fp32r (1 row/cycle at free-size 512); accumulation
    stays fp32 so the error stays ~3e-3 relative. The PE is the hard
    floor: 128x128 MACs/cycle at ~2.37GHz effective = ~450us for the
    1.065M-row stream; fp8 modes don't raise MACs/cycle on trn2.
  - Startup is pipelined: x and bias DMAs issue on the Activation HWDGE
    queue while W streams on the SP queue (parallel DGE issue), the fold
    runs per-k-tile with its DVE adds 3-deep in PSUM, and the t=0..2
    main matmul chains trail 3 k-tiles behind the fold so the PE never
    waits on the PE->DVE->PE fold round trip. x DMAs stay 4 tiles ahead
    of use so transient HBM contention never idles (and re-throttles)
    the PE.
  - psum layout is [128 tokens, 512 out]; bias is added during PSUM
    eviction; output rows land directly in [tokens, out_shard] layout so
    the host-side gather is a plain concatenate.

Host-side work is layout only: pack x as [p, T, a, t] (so each token-tile
DMA is 128 partitions x 8KB contiguous), pre-transpose W/B slices, and
broadcast bias; then concatenate the 8 output shards.
"""

import numpy as np
from ml_dtypes import bfloat16

B_DIM, S_DIM = 4, 2048
IN_F = 4096
OUT_F = 4096
RANK = 16
N_CORES = 8
O_SHARD = OUT_F // N_CORES          # 512
TOK = B_DIM * S_DIM                 # 8192
T_TILES = TOK // 128                # 64
K_TILES = IN_F // 128               # 32
N_XBUF = 6                          # x-tile pool bufs
X_PF = 4                            # x DMA issue distance ahead of use

_CACHE = {}
LAST_RESULTS = None  # test harness introspection


def _build_nc():
    import concourse.mybir as mybir
    import concourse.tile as tile
    from concourse import bacc

    nc = bacc.Bacc("TRN2", target_bir_lowering=False)
    f32 = mybir.dt.float32
    bf16 = mybir.dt.bfloat16

    x_d = nc.dram_tensor("x_re", (128, T_TILES, K_TILES, 128), bf16,
                         kind="ExternalInput")
    w_d = nc.dram_tensor("w_re", (128, K_TILES, O_SHARD), bf16,
                         kind="ExternalInput")
    a_d = nc.dram_tensor("a_t", (RANK, IN_F), bf16, kind="ExternalInput")
    bt_d = nc.dram_tensor("b_t", (RANK, O_SHARD), bf16, kind="ExternalInput")
    bias_d = nc.dram_tensor("bias_b", (128, O_SHARD), f32,
                            kind="ExternalInput")
    y_d = nc.dram_tensor("y", (TOK, O_SHARD), bf16, kind="ExternalOutput")

    with tile.TileContext(nc) as tc:
        with (
            tc.tile_pool(name="wpool", bufs=1) as wpool,
            tc.tile_pool(name="const", bufs=1) as const,
            tc.tile_pool(name="xpool", bufs=N_XBUF) as xpool,
            tc.tile_pool(name="opool", bufs=3) as opool,
            tc.tile_pool(name="psum", bufs=4, space="PSUM") as psum_pool,
            tc.tile_pool(name="psumF", bufs=3, space="PSUM") as psumF,
        ):
            # A arrives in two half-DMAs on the two queues: the fold's
            # first 16 k-tiles only need the low half, so the first fold
            # matmul gates ~4us earlier than a single 8KB/partition A DMA.
            a_sb = const.tile([RANK, IN_F], bf16)
            nc.scalar.dma_start(a_sb[:, :IN_F // 2], a_d[:, :IN_F // 2])
            b_sb = const.tile([RANK, O_SHARD], bf16)
            nc.sync.dma_start(b_sb[:], bt_d[:])

            # Two parallel DMA issue queues: W (and y) on the SP sequencer,
            # x and bias on the Activation HWDGE. Each dma_start costs
            # ~0.6us of sequencer issue time, so splitting the streams
            # halves the serial issue latency at startup.
            N_PRE = 3
            x_tiles = [None] * T_TILES
            for t in range(N_PRE):
                x_sb = xpool.tile([128, K_TILES, 128], bf16, name=f"x{t}",
                                  tag="x")
                nc.scalar.dma_start(x_sb[:], x_d[:, t, :, :])
                x_tiles[t] = x_sb

            bias_sb = const.tile([128, O_SHARD], f32)
            nc.scalar.dma_start(bias_sb[:], bias_d[:])

            # Per-k-tile W tiles: individual tiles keep the DVE fold writes
            # and the trailing matmul reads on disjoint tiles (no aliasing
            # through a shared chunk tile's dependency tracking).
            w_sb = []
            for a in range(K_TILES):
                w_t = wpool.tile([128, O_SHARD], bf16, name=f"w{a}",
                                 tag=f"w{a}")
                nc.sync.dma_start(w_t[:], w_d[:, a, :])
                w_sb.append(w_t)
                if a == 8:
                    # high half of A, needed from fold k-tile 16 on
                    nc.sync.dma_start(a_sb[:, IN_F // 2:],
                                      a_d[:, IN_F // 2:])

            def w_eff(a):
                return w_sb[a][:]

            # Fold the LoRA delta into W in place, with the t=0..2 main
            # matmul chains trailing LAG k-tiles behind so the PE never
            # waits on the fold's PE->DVE->PE round trip:
            #   w_eff[a] += A[:, a*128:(a+1)*128]^T @ B^T
            LAG = 3
            pt_pre = [psum_pool.tile([128, O_SHARD], f32, name=f"pt{t}",
                                     tag="pt")
                      for t in range(N_PRE)]

            def pre_mm(a):
                for t in range(N_PRE):
                    nc.tensor.matmul(
                        pt_pre[t][:],
                        x_tiles[t][:, a, :],
                        w_eff(a),
                        start=(a == 0), stop=(a == K_TILES - 1),
                    )

            for a in range(K_TILES):
                pd = psumF.tile([128, O_SHARD], f32)
                nc.tensor.matmul(
                    pd[:],
                    a_sb[:, a * 128:(a + 1) * 128],
                    b_sb[:],
                    start=True, stop=True,
                )
                nc.vector.tensor_add(w_sb[a][:], w_sb[a][:], pd[:])
                if a >= LAG:
                    pre_mm(a - LAG)
            for a in range(K_TILES - LAG, K_TILES):
                pre_mm(a)
            for t in range(N_PRE):
                o_sb = opool.tile([128, O_SHARD], bf16, name=f"o{t}",
                                  tag="o")
                nc.vector.tensor_add(o_sb[:], pt_pre[t][:], bias_sb[:])
                nc.sync.dma_start(y_d[t * 128:(t + 1) * 128, :], o_sb[:])

            # Main GEMM: psum[128t, 512o] = sum_a x_tile_a^T @ w_eff_a
            # x DMAs issue X_PF tiles ahead of use so a transient HBM
            # hiccup never idles the PE long enough to re-throttle HAM.
            for t in range(N_PRE, T_TILES):
                for tp in range(t, min(t + X_PF, T_TILES)):
                    if x_tiles[tp] is None:
                        x_sb = xpool.tile([128, K_TILES, 128], bf16,
                                          name="x", tag="x")
                        nc.scalar.dma_start(x_sb[:], x_d[:, tp, :, :])
                        x_tiles[tp] = x_sb
                pt = psum_pool.tile([128, O_SHARD], f32, name="pt",
                                    tag="pt")
                for a in range(K_TILES):
                    nc.tensor.matmul(
                        pt[:],
                        x_tiles[t][:, a, :],
                        w_eff(a),
                        start=(a == 0), stop=(a == K_TILES - 1),
                    )
                o_sb = opool.tile([128, O_SHARD], bf16, name="o", tag="o")
                nc.vector.tensor_add(o_sb[:], pt[:], bias_sb[:])
                nc.sync.dma_start(y_d[t * 128:(t + 1) * 128, :], o_sb[:])

    nc.compile()
    return nc


def _pack_x(x):
    x2 = np.asarray(x, dtype=np.float32).reshape(TOK, IN_F).astype(bfloat16)
    # x_re[p, T, a, t] = x2[T*128 + t, a*128 + p]
    xr = x2.reshape(T_TILES, 128, K_TILES, 128)      # (T, t, a, p)
    return np.ascontiguousarray(xr.transpose(3, 0, 2, 1))


def kernel(x, weight, A, B, bias):
    global LAST_RESULTS
    from concourse.bass_utils import run_bass_kernel_spmd

    if "nc" not in _CACHE:
        _CACHE["nc"] = _build_nc()
    nc = _CACHE["nc"]

    weight = np.asarray(weight, dtype=np.float32)
    A = np.asarray(A, dtype=np.float32)
    B = np.asarray(B, dtype=np.float32)
    bias = np.asarray(bias, dtype=np.float32)

    x_re = _pack_x(x)
    a_t = np.ascontiguousarray(A.astype(bfloat16))

    in_maps = []
    for c in range(N_CORES):
        sl = slice(c * O_SHARD, (c + 1) * O_SHARD)
        w_s = weight[sl].astype(bfloat16)             # (512, 4096)
        # w_re[p, a, o] = w_s[o, a*128 + p]
        w_re = np.ascontiguousarray(
            w_s.T.reshape(K_TILES, 128, O_SHARD).transpose(1, 0, 2))
        b_t = np.ascontiguousarray(B[sl].T.astype(bfloat16))   # (16, 512)
        bias_b = np.ascontiguousarray(
            np.broadcast_to(bias[sl], (128, O_SHARD)))
        in_maps.append({
            "x_re": x_re,
            "w_re": w_re,
            "a_t": a_t,
            "b_t": b_t,
            "bias_b": bias_b,
        })

    res = run_bass_kernel_spmd(nc, in_maps, core_ids=list(range(N_CORES)))
    LAST_RESULTS = res

    y = np.concatenate(
        [res.results[c]["y"].astype(np.float32) for c in range(N_CORES)],
        axis=1)
    return y.reshape(B_DIM, S_DIM, OUT_F)
